# revision 6
# baseline (speedup 1.0000x reference)
"""Per-sample dynamic-filter Conv2D (VALID, stride 1) on 8 Trainium2 NeuronCores.

Problem: X [16,128,128,128] (NHWC) conv with per-sample filters
kernel [16,3,3,128,128] (HWIO) -> out [16,126,126,128].

Sharding: pure data parallel - 2 samples per core, no communication.

Design (v2): all layout transforms happen on the host so the device does
nothing but conv matmuls:
  - Host pre-transposes X to X^T [Cin, H*W] (bf16, zero-padded past HW so
    tap-shifted windows stay in bounds) and pre-flattens the filter to
    [Cin, tap*Cout] (bf16).
  - Device, per 504-column output tile: 9 PSUM-accumulated bf16 matmuls
    (lhsT = filter tap [ci, co], rhs = X^T window [ci, 504]), then a
    PSUM->SBUF copy (alternating DVE/ACT) and a contiguous DMA to DRAM
    out^T [co, p] (full-width rows: p = oh*W + ow, garbage at ow >= OW).
  - Host slices off the ow >= OW columns and transposes out^T back to NHWC.
bf16 inputs with f32 PSUM accumulation give rel err ~2e-3 (gate: 2e-2).
"""

import sys

_BASS_PATH = "/opt/trn_rl_repo"
if _BASS_PATH not in sys.path:
    sys.path.insert(0, _BASS_PATH)

import numpy as np

import concourse.mybir as mybir  # noqa: E402
import concourse.tile as tile  # noqa: E402
from concourse import bacc  # noqa: E402

F32 = mybir.dt.float32
BF16 = mybir.dt.bfloat16

# Full-problem constants
B, H, W, CIN, COUT, KH, KW = 16, 128, 128, 128, 128, 3, 3
N_CORES = 8
S = B // N_CORES  # samples per core
P = 128
OH, OW = H - KH + 1, W - KW + 1
HW = H * W                      # input positions (16384)
NHW = OH * W                    # full-width output positions (16128)
HALO = (KH - 1) * W + (KW - 1)  # max tap offset (258)
XT_COLS = ((HW + HALO + P - 1) // P) * P  # padded X^T columns (16640)
NTILE = 504                     # output tile columns (NHW % NTILE == 0)
NT = NHW // NTILE               # 32 tiles per sample


def build_conv_nc(n_tile=NTILE):
    """Build the per-core Bass program. Returns compiled nc."""
    assert NHW % n_tile == 0 and n_tile <= 512
    nt = NHW // n_tile
    nc = bacc.Bacc("TRN2", target_bir_lowering=False, debug=False)
    xd = nc.dram_tensor("xt", [S, CIN, XT_COLS], BF16, kind="ExternalInput").ap()
    kd = nc.dram_tensor(
        "k", [S, CIN, KH * KW * COUT], BF16, kind="ExternalInput"
    ).ap()
    od = nc.dram_tensor("o", [S, COUT, NHW], F32, kind="ExternalOutput").ap()

    # X^T DMA chunk column boundaries: a small first chunk so tile 0's
    # data lands early, bigger chunks after.
    import os

    ch0 = int(os.environ.get("CONV_CH0", "768"))
    nwu = int(os.environ.get("CONV_WARMUP", "11"))
    bounds = [0, ch0]
    rest = XT_COLS - ch0
    nrest = 6
    step = ((rest // nrest) // 16 + 1) * 16
    while bounds[-1] < XT_COLS:
        bounds.append(min(bounds[-1] + step, XT_COLS))

    with tile.TileContext(nc) as tc:
        with (
            tc.tile_pool(name="xt", bufs=2) as xt_pool,
            tc.tile_pool(name="filt", bufs=2) as filt_pool,
            tc.tile_pool(name="wusrc", bufs=1) as wusrc_pool,
            tc.tile_pool(name="ostage", bufs=8) as ostage_pool,
            tc.tile_pool(name="acc", bufs=6, space="PSUM") as acc_pool,
            tc.tile_pool(name="wu", bufs=1, space="PSUM") as wu_pool,
        ):
            state = {}
            for s in range(S):
                filt = filt_pool.tile(
                    [P, KH * KW * COUT], BF16, tag=f"filt{s}", name=f"filt{s}"
                )
                xt = xt_pool.tile([P, XT_COLS], BF16, tag=f"xt{s}", name=f"xt{s}")
                state[s] = (filt, xt)
            filt0, xt0 = state[0]
            # PE warm-up source first (GpSimd is the earliest-free engine),
            # then the critical first X^T chunk on GpSimd's DMA path —
            # its doorbell lands ~1 us before the Sync queue is ready.
            wsrc = None
            if nwu:
                wsrc = wusrc_pool.tile([P, NTILE], BF16, tag="wsrc", name="wsrc")
                nc.gpsimd.memset(wsrc[:], 1.0)
            nc.gpsimd.dma_start(
                out=xt0[:, 0 : bounds[1]], in_=xd[0, :, 0 : bounds[1]]
            )
            # Filter for sample 0 heads the Sync queue; all bulk loads queue
            # FIFO behind it so they cannot race the critical pair.
            nc.sync.dma_start(out=filt0[:], in_=kd[0])
            # PE warm-up: matmuls on the memset tile (no DMA dependency)
            # keep the PE busy from the end of the preamble so the HAM
            # clock gate is at 8/8 when the real stream starts.
            if nwu:
                wu = wu_pool.tile([P, NTILE], F32, tag="wu", name="wu")
                for _ in range(nwu):
                    nc.tensor.matmul(
                        wu[:],
                        wsrc[:, 0:COUT],
                        wsrc[:],
                        start=True,
                        stop=True,
                    )
            for s in range(S):
                filt, xt = state[s]
                for c in range(len(bounds) - 1):
                    if s == 0 and c == 0:
                        continue
                    nc.sync.dma_start(
                        out=xt[:, bounds[c] : bounds[c + 1]],
                        in_=xd[s, :, bounds[c] : bounds[c + 1]],
                    )
                if s > 0:
                    nc.sync.dma_start(out=filt[:], in_=kd[s])

            for s in range(S):
                filt, xt = state[s]
                for t in range(nt):
                    base = t * n_tile
                    acc = acc_pool.tile([P, n_tile], F32, tag="acc", name="acc")
                    for tap in range(KH * KW):
                        dy, dx = divmod(tap, KW)
                        off = base + dy * W + dx
                        nc.tensor.matmul(
                            acc[:],
                            filt[:, tap * COUT : (tap + 1) * COUT],
                            xt[:, off : off + n_tile],
                            start=(tap == 0),
                            stop=(tap == KH * KW - 1),
                        )
                    o = ostage_pool.tile([P, n_tile], F32, tag="o", name="o")
                    if t % 2 == 0:
                        nc.vector.tensor_copy(o[:], acc[:])
                    else:
                        nc.scalar.copy(o[:], acc[:])
                    nc.sync.dma_start(
                        out=od[s, :, base : base + n_tile], in_=o[:]
                    )

    nc.compile()
    return nc


_NC_CACHE = {}


def _get_nc():
    import os

    n_tile = int(os.environ.get("CONV_NTILE", str(NTILE)))
    if n_tile not in _NC_CACHE:
        _NC_CACHE[n_tile] = build_conv_nc(n_tile=n_tile)
    return _NC_CACHE[n_tile]


def make_in_maps(X, K):
    """Host-side prep: X^T (padded, bf16) + flattened filters (bf16)."""
    import ml_dtypes

    bf = ml_dtypes.bfloat16
    X = np.asarray(X, dtype=np.float32)
    K = np.asarray(K, dtype=np.float32)
    assert X.shape == (B, H, W, CIN), X.shape
    assert K.shape == (B, KH, KW, CIN, COUT), K.shape
    Xt = np.zeros((B, CIN, XT_COLS), dtype=bf)
    Xt[:, :, :HW] = X.reshape(B, HW, CIN).transpose(0, 2, 1).astype(bf)
    # [B, kh, kw, ci, co] -> [B, ci, kh*kw*co]
    Kt = np.ascontiguousarray(
        K.transpose(0, 3, 1, 2, 4).reshape(B, CIN, KH * KW * COUT).astype(bf)
    )
    return [
        {"xt": Xt[i * S : (i + 1) * S], "k": Kt[i * S : (i + 1) * S]}
        for i in range(N_CORES)
    ]


def unpack_output(results):
    """[S, COUT, NHW] f32 per core -> full [B, OH, OW, COUT] f32."""
    out_t = np.empty((B, COUT, NHW), dtype=np.float32)
    for i in range(N_CORES):
        out_t[i * S : (i + 1) * S] = results[i]["o"]
    # [B, co, oh, W] -> drop ow >= OW -> [B, oh, ow, co]
    return np.ascontiguousarray(
        out_t.reshape(B, COUT, OH, W)[:, :, :, :OW].transpose(0, 2, 3, 1)
    )


def kernel(**inputs):
    from concourse.bass_utils import run_bass_kernel_spmd

    nc = _get_nc()
    in_maps = make_in_maps(inputs["X"], inputs["kernel"])
    res = run_bass_kernel_spmd(nc, in_maps, list(range(N_CORES)))
    return unpack_output(res.results)


# revision 9
# speedup vs baseline: 1.0174x; 1.0174x over previous
"""Per-sample dynamic-filter Conv2D (VALID, stride 1) on 8 Trainium2 NeuronCores.

Problem: X [16,128,128,128] (NHWC) conv with per-sample filters
kernel [16,3,3,128,128] (HWIO) -> out [16,126,126,128].

Sharding: pure data parallel - 2 samples per core, no communication.

Design (v2): all layout transforms happen on the host so the device does
nothing but conv matmuls:
  - Host pre-transposes X to X^T [Cin, H*W] (bf16, zero-padded past HW so
    tap-shifted windows stay in bounds) and pre-flattens the filter to
    [Cin, tap*Cout] (bf16).
  - Device, per 504-column output tile: 9 PSUM-accumulated bf16 matmuls
    (lhsT = filter tap [ci, co], rhs = X^T window [ci, 504]), then a
    PSUM->SBUF copy (alternating DVE/ACT) and a contiguous DMA to DRAM
    out^T [co, p] (full-width rows: p = oh*W + ow, garbage at ow >= OW).
  - Host slices off the ow >= OW columns and transposes out^T back to NHWC.
bf16 inputs with f32 PSUM accumulation give rel err ~2e-3 (gate: 2e-2).
"""

import sys

_BASS_PATH = "/opt/trn_rl_repo"
if _BASS_PATH not in sys.path:
    sys.path.insert(0, _BASS_PATH)

import numpy as np

import concourse.mybir as mybir  # noqa: E402
import concourse.tile as tile  # noqa: E402
from concourse import bacc  # noqa: E402

F32 = mybir.dt.float32
BF16 = mybir.dt.bfloat16

# Full-problem constants
B, H, W, CIN, COUT, KH, KW = 16, 128, 128, 128, 128, 3, 3
N_CORES = 8
S = B // N_CORES  # samples per core
P = 128
OH, OW = H - KH + 1, W - KW + 1
HW = H * W                      # input positions (16384)
NHW = OH * W                    # full-width output positions (16128)
HALO = (KH - 1) * W + (KW - 1)  # max tap offset (258)
XT_COLS = ((HW + HALO + P - 1) // P) * P  # padded X^T columns (16640)
NTILE = 504                     # output tile columns (NHW % NTILE == 0)
NT = NHW // NTILE               # 32 tiles per sample


ROWS = 3  # output rows per tile in valid-only mode (378-column tiles)


def build_conv_nc(n_tile=NTILE, valid=False):
    """Build the per-core Bass program. Returns compiled nc."""
    assert NHW % n_tile == 0 and n_tile <= 512
    if valid:
        nt = OH // ROWS          # 42 tiles per sample
        n_out = OH * OW          # 15876 valid positions
    else:
        nt = NHW // n_tile
        n_out = NHW
    nc = bacc.Bacc("TRN2", target_bir_lowering=False, debug=False)
    xd = nc.dram_tensor("xt", [S, CIN, XT_COLS], BF16, kind="ExternalInput").ap()
    kd = nc.dram_tensor(
        "k", [S, CIN, KH * KW * COUT], BF16, kind="ExternalInput"
    ).ap()
    od = nc.dram_tensor("o", [S, COUT, n_out], F32, kind="ExternalOutput").ap()

    # X^T DMA chunk column boundaries: a small first chunk so tile 0's
    # data lands early, bigger chunks after.
    import os

    ch0 = int(os.environ.get("CONV_CH0", "768"))
    nwu = int(os.environ.get("CONV_WARMUP", "11"))
    bounds = [0, ch0]
    rest = XT_COLS - ch0
    nrest = 6
    step = ((rest // nrest) // 16 + 1) * 16
    while bounds[-1] < XT_COLS:
        bounds.append(min(bounds[-1] + step, XT_COLS))

    with tile.TileContext(nc) as tc:
        with (
            tc.tile_pool(name="xt", bufs=2) as xt_pool,
            tc.tile_pool(name="filt", bufs=2) as filt_pool,
            tc.tile_pool(name="wusrc", bufs=1) as wusrc_pool,
            tc.tile_pool(name="ostage", bufs=8) as ostage_pool,
            tc.tile_pool(name="acc", bufs=6, space="PSUM") as acc_pool,
            tc.tile_pool(name="wu", bufs=1, space="PSUM") as wu_pool,
        ):
            state = {}
            for s in range(S):
                filt = filt_pool.tile(
                    [P, KH * KW * COUT], BF16, tag=f"filt{s}", name=f"filt{s}"
                )
                xt = xt_pool.tile([P, XT_COLS], BF16, tag=f"xt{s}", name=f"xt{s}")
                state[s] = (filt, xt)
            filt0, xt0 = state[0]
            wsrc = None
            if nwu:
                wsrc = wusrc_pool.tile([P, NTILE], BF16, tag="wsrc", name="wsrc")
                nc.gpsimd.memset(wsrc[:], 1.0)
            # Critical pair heads the Sync queue; all bulk loads queue FIFO
            # behind it on the same HWDGE ring so they cannot race it.
            nc.sync.dma_start(out=filt0[:], in_=kd[0])
            nc.sync.dma_start(
                out=xt0[:, 0 : bounds[1]], in_=xd[0, :, 0 : bounds[1]]
            )
            # PE warm-up: matmuls on the memset tile (no DMA dependency)
            # keep the PE busy from the end of the preamble so the HAM
            # clock gate is at 8/8 when the real stream starts.
            if nwu:
                wu = wu_pool.tile([P, NTILE], F32, tag="wu", name="wu")
                for _ in range(nwu):
                    nc.tensor.matmul(
                        wu[:],
                        wsrc[:, 0:COUT],
                        wsrc[:],
                        start=True,
                        stop=True,
                    )
            for s in range(S):
                filt, xt = state[s]
                for c in range(len(bounds) - 1):
                    if s == 0 and c == 0:
                        continue
                    nc.sync.dma_start(
                        out=xt[:, bounds[c] : bounds[c + 1]],
                        in_=xd[s, :, bounds[c] : bounds[c + 1]],
                    )
                if s > 0:
                    nc.sync.dma_start(out=filt[:], in_=kd[s])

            ntile_v = ROWS * OW  # 378
            for s in range(S):
                filt, xt = state[s]
                for t in range(nt):
                    if valid:
                        r0 = t * ROWS
                        base = r0 * OW
                        cols = ntile_v
                        acc = acc_pool.tile([P, cols], F32, tag="acc", name="acc")
                        acc_mm = acc[:].rearrange("p (r w) -> p r w", r=ROWS)
                    else:
                        base = t * n_tile
                        cols = n_tile
                        acc = acc_pool.tile([P, cols], F32, tag="acc", name="acc")
                        acc_mm = acc[:]
                    for tap in range(KH * KW):
                        dy, dx = divmod(tap, KW)
                        if valid:
                            b = (r0 + dy) * W + dx
                            rhs = xt[:, b : b + ROWS * W].rearrange(
                                "ci (r w) -> ci r w", r=ROWS
                            )[:, :, :OW]
                        else:
                            off = base + dy * W + dx
                            rhs = xt[:, off : off + n_tile]
                        nc.tensor.matmul(
                            acc_mm,
                            filt[:, tap * COUT : (tap + 1) * COUT],
                            rhs,
                            start=(tap == 0),
                            stop=(tap == KH * KW - 1),
                        )
                    o = ostage_pool.tile([P, cols], F32, tag="o", name="o")
                    if t % 2 == 0:
                        nc.vector.tensor_copy(o[:], acc[:])
                    else:
                        nc.scalar.copy(o[:], acc[:])
                    nc.sync.dma_start(
                        out=od[s, :, base : base + cols], in_=o[:]
                    )

    nc.compile()
    return nc


_NC_CACHE = {}


def _get_nc():
    import os

    n_tile = int(os.environ.get("CONV_NTILE", str(NTILE)))
    if n_tile not in _NC_CACHE:
        _NC_CACHE[n_tile] = build_conv_nc(n_tile=n_tile)
    return _NC_CACHE[n_tile]


def make_in_maps(X, K):
    """Host-side prep: X^T (padded, bf16) + flattened filters (bf16)."""
    import ml_dtypes

    bf = ml_dtypes.bfloat16
    X = np.asarray(X, dtype=np.float32)
    K = np.asarray(K, dtype=np.float32)
    assert X.shape == (B, H, W, CIN), X.shape
    assert K.shape == (B, KH, KW, CIN, COUT), K.shape
    Xt = np.zeros((B, CIN, XT_COLS), dtype=bf)
    Xt[:, :, :HW] = X.reshape(B, HW, CIN).transpose(0, 2, 1).astype(bf)
    # [B, kh, kw, ci, co] -> [B, ci, kh*kw*co]
    Kt = np.ascontiguousarray(
        K.transpose(0, 3, 1, 2, 4).reshape(B, CIN, KH * KW * COUT).astype(bf)
    )
    return [
        {"xt": Xt[i * S : (i + 1) * S], "k": Kt[i * S : (i + 1) * S]}
        for i in range(N_CORES)
    ]


def unpack_output(results):
    """[S, COUT, NHW] f32 per core -> full [B, OH, OW, COUT] f32."""
    out_t = np.empty((B, COUT, NHW), dtype=np.float32)
    for i in range(N_CORES):
        out_t[i * S : (i + 1) * S] = results[i]["o"]
    # [B, co, oh, W] -> drop ow >= OW -> [B, oh, ow, co]
    return np.ascontiguousarray(
        out_t.reshape(B, COUT, OH, W)[:, :, :, :OW].transpose(0, 2, 3, 1)
    )


def kernel(**inputs):
    from concourse.bass_utils import run_bass_kernel_spmd

    nc = _get_nc()
    in_maps = make_in_maps(inputs["X"], inputs["kernel"])
    res = run_bass_kernel_spmd(nc, in_maps, list(range(N_CORES)))
    return unpack_output(res.results)


# revision 12
# speedup vs baseline: 1.0178x; 1.0004x over previous
"""Per-sample dynamic-filter Conv2D (VALID, stride 1) on 8 Trainium2 NeuronCores.

Problem: X [16,128,128,128] (NHWC) conv with per-sample filters
kernel [16,3,3,128,128] (HWIO) -> out [16,126,126,128].

Sharding: pure data parallel - 2 samples per core, no communication.

Design (v2): all layout transforms happen on the host so the device does
nothing but conv matmuls:
  - Host pre-transposes X to X^T [Cin, H*W] (bf16, zero-padded past HW so
    tap-shifted windows stay in bounds) and pre-flattens the filter to
    [Cin, tap*Cout] (bf16).
  - Device, per 504-column output tile: 9 PSUM-accumulated bf16 matmuls
    (lhsT = filter tap [ci, co], rhs = X^T window [ci, 504]), then a
    PSUM->SBUF copy (alternating DVE/ACT) and a contiguous DMA to DRAM
    out^T [co, p] (full-width rows: p = oh*W + ow, garbage at ow >= OW).
  - Host slices off the ow >= OW columns and transposes out^T back to NHWC.
bf16 inputs with f32 PSUM accumulation give rel err ~2e-3 (gate: 2e-2).
"""

import sys

_BASS_PATH = "/opt/trn_rl_repo"
if _BASS_PATH not in sys.path:
    sys.path.insert(0, _BASS_PATH)

import numpy as np

import concourse.mybir as mybir  # noqa: E402
import concourse.tile as tile  # noqa: E402
from concourse import bacc  # noqa: E402

F32 = mybir.dt.float32
BF16 = mybir.dt.bfloat16

# Full-problem constants
B, H, W, CIN, COUT, KH, KW = 16, 128, 128, 128, 128, 3, 3
N_CORES = 8
S = B // N_CORES  # samples per core
P = 128
OH, OW = H - KH + 1, W - KW + 1
HW = H * W                      # input positions (16384)
NHW = OH * W                    # full-width output positions (16128)
HALO = (KH - 1) * W + (KW - 1)  # max tap offset (258)
XT_COLS = ((HW + HALO + P - 1) // P) * P  # padded X^T columns (16640)
NTILE = 504                     # output tile columns (NHW % NTILE == 0)
NT = NHW // NTILE               # 32 tiles per sample


ROWS = 3  # output rows per tile in valid-only mode (378-column tiles)


def build_conv_nc(n_tile=NTILE, valid=False):
    """Build the per-core Bass program. Returns compiled nc."""
    assert NHW % n_tile == 0 and n_tile <= 512
    if valid:
        nt = OH // ROWS          # 42 tiles per sample
        n_out = OH * OW          # 15876 valid positions
    else:
        nt = NHW // n_tile
        n_out = NHW
    nc = bacc.Bacc("TRN2", target_bir_lowering=False, debug=False)
    xd = nc.dram_tensor("xt", [S, CIN, XT_COLS], BF16, kind="ExternalInput").ap()
    kd = nc.dram_tensor(
        "k", [S, CIN, KH * KW * COUT], BF16, kind="ExternalInput"
    ).ap()
    od = nc.dram_tensor("o", [S, COUT, n_out], F32, kind="ExternalOutput").ap()

    # X^T DMA chunk column boundaries: a small first chunk so tile 0's
    # data lands early, bigger chunks after.
    import os

    ch0 = int(os.environ.get("CONV_CH0", "1040"))
    nwu = int(os.environ.get("CONV_WARMUP", "11"))
    bounds = [0, ch0]
    rest = XT_COLS - ch0
    nrest = 6
    step = ((rest // nrest) // 16 + 1) * 16
    while bounds[-1] < XT_COLS:
        bounds.append(min(bounds[-1] + step, XT_COLS))

    with tile.TileContext(nc) as tc:
        with (
            tc.tile_pool(name="xt", bufs=2) as xt_pool,
            tc.tile_pool(name="filt", bufs=2) as filt_pool,
            tc.tile_pool(name="wusrc", bufs=1) as wusrc_pool,
            tc.tile_pool(name="ostage", bufs=8) as ostage_pool,
            tc.tile_pool(name="acc", bufs=6, space="PSUM") as acc_pool,
            tc.tile_pool(name="wu", bufs=1, space="PSUM") as wu_pool,
        ):
            state = {}
            for s in range(S):
                filt = filt_pool.tile(
                    [P, KH * KW * COUT], BF16, tag=f"filt{s}", name=f"filt{s}"
                )
                xt = xt_pool.tile([P, XT_COLS], BF16, tag=f"xt{s}", name=f"xt{s}")
                state[s] = (filt, xt)
            filt0, xt0 = state[0]
            wsrc = None
            if nwu:
                wsrc = wusrc_pool.tile([P, NTILE], BF16, tag="wsrc", name="wsrc")
                nc.gpsimd.memset(wsrc[:], 1.0)
            # Critical pair heads the Sync queue; all bulk loads queue FIFO
            # behind it on the same HWDGE ring so they cannot race it.
            nc.sync.dma_start(out=filt0[:], in_=kd[0])
            nc.sync.dma_start(
                out=xt0[:, 0 : bounds[1]], in_=xd[0, :, 0 : bounds[1]]
            )
            # PE warm-up: matmuls on the memset tile (no DMA dependency)
            # keep the PE busy from the end of the preamble so the HAM
            # clock gate is at 8/8 when the real stream starts.
            if nwu:
                wu = wu_pool.tile([P, NTILE], F32, tag="wu", name="wu")
                for _ in range(nwu):
                    nc.tensor.matmul(
                        wu[:],
                        wsrc[:, 0:COUT],
                        wsrc[:],
                        start=True,
                        stop=True,
                    )
            for s in range(S):
                filt, xt = state[s]
                for c in range(len(bounds) - 1):
                    if s == 0 and c == 0:
                        continue
                    nc.sync.dma_start(
                        out=xt[:, bounds[c] : bounds[c + 1]],
                        in_=xd[s, :, bounds[c] : bounds[c + 1]],
                    )
                if s > 0:
                    nc.sync.dma_start(out=filt[:], in_=kd[s])

            ntile_v = ROWS * OW  # 378
            for s in range(S):
                filt, xt = state[s]
                for t in range(nt):
                    if valid:
                        r0 = t * ROWS
                        base = r0 * OW
                        cols = ntile_v
                        acc = acc_pool.tile([P, cols], F32, tag="acc", name="acc")
                        acc_mm = acc[:].rearrange("p (r w) -> p r w", r=ROWS)
                    else:
                        base = t * n_tile
                        cols = n_tile
                        acc = acc_pool.tile([P, cols], F32, tag="acc", name="acc")
                        acc_mm = acc[:]
                    for tap in range(KH * KW):
                        dy, dx = divmod(tap, KW)
                        if valid:
                            b = (r0 + dy) * W + dx
                            rhs = xt[:, b : b + ROWS * W].rearrange(
                                "ci (r w) -> ci r w", r=ROWS
                            )[:, :, :OW]
                        else:
                            off = base + dy * W + dx
                            rhs = xt[:, off : off + n_tile]
                        nc.tensor.matmul(
                            acc_mm,
                            filt[:, tap * COUT : (tap + 1) * COUT],
                            rhs,
                            start=(tap == 0),
                            stop=(tap == KH * KW - 1),
                        )
                    o = ostage_pool.tile([P, cols], F32, tag="o", name="o")
                    if t % 2 == 0:
                        nc.vector.tensor_copy(o[:], acc[:])
                    else:
                        nc.scalar.copy(o[:], acc[:])
                    nc.sync.dma_start(
                        out=od[s, :, base : base + cols], in_=o[:]
                    )

    nc.compile()
    return nc


_NC_CACHE = {}


def _valid_mode():
    import os

    return os.environ.get("CONV_VALID", "0") == "1"


def _get_nc():
    import os

    n_tile = int(os.environ.get("CONV_NTILE", str(NTILE)))
    key = (n_tile, _valid_mode())
    if key not in _NC_CACHE:
        _NC_CACHE[key] = build_conv_nc(n_tile=n_tile, valid=key[1])
    return _NC_CACHE[key]


def make_in_maps(X, K):
    """Host-side prep: X^T (padded, bf16) + flattened filters (bf16)."""
    import ml_dtypes

    bf = ml_dtypes.bfloat16
    X = np.asarray(X, dtype=np.float32)
    K = np.asarray(K, dtype=np.float32)
    assert X.shape == (B, H, W, CIN), X.shape
    assert K.shape == (B, KH, KW, CIN, COUT), K.shape
    Xt = np.zeros((B, CIN, XT_COLS), dtype=bf)
    Xt[:, :, :HW] = X.reshape(B, HW, CIN).transpose(0, 2, 1).astype(bf)
    # [B, kh, kw, ci, co] -> [B, ci, kh*kw*co]
    Kt = np.ascontiguousarray(
        K.transpose(0, 3, 1, 2, 4).reshape(B, CIN, KH * KW * COUT).astype(bf)
    )
    return [
        {"xt": Xt[i * S : (i + 1) * S], "k": Kt[i * S : (i + 1) * S]}
        for i in range(N_CORES)
    ]


def unpack_output(results):
    """[S, COUT, n_out] f32 per core -> full [B, OH, OW, COUT] f32."""
    if _valid_mode():
        out_t = np.empty((B, COUT, OH * OW), dtype=np.float32)
        for i in range(N_CORES):
            out_t[i * S : (i + 1) * S] = results[i]["o"]
        return np.ascontiguousarray(
            out_t.reshape(B, COUT, OH, OW).transpose(0, 2, 3, 1)
        )
    out_t = np.empty((B, COUT, NHW), dtype=np.float32)
    for i in range(N_CORES):
        out_t[i * S : (i + 1) * S] = results[i]["o"]
    # [B, co, oh, W] -> drop ow >= OW -> [B, oh, ow, co]
    return np.ascontiguousarray(
        out_t.reshape(B, COUT, OH, W)[:, :, :, :OW].transpose(0, 2, 3, 1)
    )


def kernel(**inputs):
    from concourse.bass_utils import run_bass_kernel_spmd

    nc = _get_nc()
    in_maps = make_in_maps(inputs["X"], inputs["kernel"])
    res = run_bass_kernel_spmd(nc, in_maps, list(range(N_CORES)))
    return unpack_output(res.results)
